# revision 1
# baseline (speedup 1.0000x reference)
"""Causal self-attention (B=4, T=2048, D=1024, H=16) on 8 NeuronCores.

Sharding: core c handles batch b=c//2 and head-group hg=c%2 (8 of 16 heads).
Per core: column-parallel Wq/Wk/Wv (512 cols), row-parallel Wo (512 rows).
Host sums the two partial outputs per batch and adds bo. No collectives.

On-chip layout (all transposed; no on-chip transposes needed):
  xT [D=1024, T=2048] (host pre-transposed), qT/kT [512 dout, T],
  V natural [T, 8 heads x (64 dv + 1 ones col)].
  Scores computed as S^T [t_k, t_q] = kT.T @ qT; exp (no max subtraction --
  scores are O(4), fp32 exp safe); PV matmul out^T[dv, t_q] = V_aug.T @ expS^T
  with the ones column yielding sumexp for free; divide via batched
  reciprocal + K=1 broadcast matmul; final projection consumes out^T
  directly as lhsT.

Dtypes: MODE="safe": qk projections + scores in float32r (fp32 rounded to
11-bit mantissa; 2 cyc/row on PE), V/PV/final chain in bf16 (1 cyc/row).
MODE="fast": everything bf16.
Diagonal k-tiles narrow their matmul/exp/mask N to the causally valid
column range (columns < o are fully masked in S^T tile at offset o).
"""

import os
from contextlib import ExitStack

import ml_dtypes
import numpy as np

import concourse.bacc as bacc
import concourse.mybir as mybir
import concourse.tile as tile
from concourse.bass_utils import run_bass_kernel_spmd

B, T, D, H, DK = 4, 2048, 1024, 16, 64
HL = 8  # heads per core
CD = HL * DK  # 512 local channels
NP = 128  # partitions
QB = 512  # query block / matmul moving dim
NDC = D // NP  # 8 din chunks
NTT = T // NP  # 16 t-tiles
NTB = T // QB  # 4 t-blocks
NPAIR = HL // 2  # 4 head pairs
F32 = mybir.dt.float32
F32R = mybir.dt.float32r
BF16 = mybir.dt.bfloat16
Exp = mybir.ActivationFunctionType.Exp
Identity = mybir.ActivationFunctionType.Identity

MODE = os.environ.get("KERNEL_MODE", "safe")

_CACHE: dict = {}


def _build_nc():
    DTQ = BF16 if MODE == "fast" else F32R  # xt / wq / wk / qt / kt / scores
    nc = bacc.Bacc("TRN2", target_bir_lowering=False, debug=False)
    xt = nc.dram_tensor("xt", [D, T], DTQ, kind="ExternalInput")
    wq = nc.dram_tensor("wq", [D, CD], DTQ, kind="ExternalInput")
    wk = nc.dram_tensor("wk", [D, CD], DTQ, kind="ExternalInput")
    wv = nc.dram_tensor("wv", [D, CD], BF16, kind="ExternalInput")
    wo = nc.dram_tensor("wo", [CD, D], BF16, kind="ExternalInput")
    bqc = nc.dram_tensor("bqc", [NP, NPAIR], F32, kind="ExternalInput")
    bkc = nc.dram_tensor("bkc", [NP, NPAIR], F32, kind="ExternalInput")
    bvr = nc.dram_tensor("bvr", [1, CD], BF16, kind="ExternalInput")
    msk = nc.dram_tensor("msk", [4, NP, QB], F32, kind="ExternalInput")
    onesd = nc.dram_tensor("onesd", [NP, QB], BF16, kind="ExternalInput")
    y = nc.dram_tensor("y", [T, D], F32, kind="ExternalOutput")

    with tile.TileContext(nc) as tc, ExitStack() as ctx:
        _body(nc, tc, ctx, DTQ, xt, wq, wk, wv, wo, bqc, bkc, bvr, msk, onesd, y)
    nc.compile()
    return nc


def _body(nc, tc, ctx, DTQ, xt, wq, wk, wv, wo, bqc, bkc, bvr, msk, onesd, y):
    const = ctx.enter_context(tc.tile_pool(name="const", bufs=1))
    vpool = ctx.enter_context(tc.tile_pool(name="v", bufs=1))
    oatp = ctx.enter_context(tc.tile_pool(name="oat", bufs=1))
    xtp = ctx.enter_context(tc.tile_pool(name="xt", bufs=9))
    # PSUM: proj(2, shared w/ bcast) + score(2 tags x 2) + pv(2 tags x 1) = 8
    projps = ctx.enter_context(tc.tile_pool(name="projps", bufs=2, space="PSUM"))
    scoreps = ctx.enter_context(tc.tile_pool(name="scoreps", bufs=2, space="PSUM"))
    pvps = ctx.enter_context(tc.tile_pool(name="pvps", bufs=1, space="PSUM"))

    # constants
    ones_t = const.tile([1, QB], BF16)
    nc.sync.dma_start(ones_t[:], onesd[0:1, :])
    bq_sb = const.tile([NP, NPAIR], F32, tag="bq")
    nc.sync.dma_start(bq_sb[:], bqc[:])
    bk_sb = const.tile([NP, NPAIR], F32, tag="bk")
    nc.sync.dma_start(bk_sb[:], bkc[:])
    bv_sb = const.tile([1, CD], BF16, tag="bv")
    nc.sync.dma_start(bv_sb[:], bvr[:])
    msk_sb = const.tile([NP, 4, QB], F32, tag="msk")
    for o in range(4):
        nc.sync.dma_start(msk_sb[:, o, :], msk[o, :, :])
    ones_f = const.tile([1, DK], F32, tag="onesf")
    nc.vector.memset(ones_f[:], 1.0)
    # warm up the exp table set early (one-time ~2.7us load overlaps V phase)
    warm = const.tile([1, 2], F32, tag="warm")
    nc.vector.memset(warm[:], 0.0)
    nc.scalar.activation(warm[:], warm[:], Exp)

    # ---- V phase: V[t, dv] for all 8 heads (bf16), with ones column ----
    v_sb = [
        vpool.tile([NP, HL, DK + 1], BF16, tag=f"v{tt}", name=f"v{tt}")
        for tt in range(NTT)
    ]
    wvp_cm = tc.tile_pool(name="wvp", bufs=1)
    wvp = wvp_cm.__enter__()
    wv_sb = wvp.tile([NP, NDC, CD], BF16, tag="wv")
    for d in range(NDC):
        nc.sync.dma_start(wv_sb[:, d, :], wv[d * NP : (d + 1) * NP, :])
    for tb in range(NTB):
        xvts = []
        for d in range(NDC):
            xvtile = wvp.tile([NP, QB], BF16, tag="xv", bufs=9, name="xv")
            src = xt[d * NP : (d + 1) * NP, tb * QB : (tb + 1) * QB]
            if DTQ == BF16:
                nc.sync.dma_start(xvtile[:], src)
            else:
                nc.gpsimd.dma_start(xvtile[:], src.bitcast(F32))  # cast f32->bf16
            xvts.append(xvtile)
        for i in range(QB // NP):
            tt = tb * (QB // NP) + i
            ps = projps.tile([NP, CD], F32, tag="proj")
            for d in range(NDC):
                nc.tensor.matmul(
                    ps[:],
                    xvts[d][:, i * NP : (i + 1) * NP],
                    wv_sb[:, d, :],
                    start=(d == 0),
                    stop=False,
                )
            # + ones_col x bv  (bias along free dim via K=1 rank-1 update)
            nc.tensor.matmul(
                ps[:], ones_t[0:1, 0:NP], bv_sb[:], start=False, stop=True
            )
            vt = v_sb[tt]
            nc.sync.dma_start(vt[:, :, DK : DK + 1], onesd[:, 0:HL])
            nc.vector.tensor_copy(vt[:, :, 0:DK], ps.rearrange("p (h k) -> p h k", h=HL))

    wvp_cm.__exit__(None, None, None)

    # ---- per head-pair: qT/kT projection then attention ----
    wqkp = ctx.enter_context(tc.tile_pool(name="wqk", bufs=2))
    qtp = ctx.enter_context(tc.tile_pool(name="qt", bufs=2))
    ktp = ctx.enter_context(tc.tile_pool(name="kt", bufs=2))
    expp = ctx.enter_context(tc.tile_pool(name="exp", bufs=2))
    smallp = ctx.enter_context(tc.tile_pool(name="small", bufs=2))
    oat = [oatp.tile([NP, T], BF16, tag=f"oat{c}", name=f"oat{c}") for c in range(NPAIR)]

    for c in range(NPAIR):
        wqc = wqkp.tile([NP, NDC, NP], DTQ, tag="wqc")
        wkc = wqkp.tile([NP, NDC, NP], DTQ, tag="wkc")
        for d in range(NDC):
            nc.sync.dma_start(
                wqc[:, d, :], wq[d * NP : (d + 1) * NP, c * NP : (c + 1) * NP]
            )
            nc.sync.dma_start(
                wkc[:, d, :], wk[d * NP : (d + 1) * NP, c * NP : (c + 1) * NP]
            )
        qt = qtp.tile([NP, T], DTQ)
        kt_t = ktp.tile([NP, T], DTQ)
        for tb in range(NTB):
            xts = []
            for d in range(NDC):
                xtile = xtp.tile([NP, QB], DTQ)
                nc.sync.dma_start(
                    xtile[:], xt[d * NP : (d + 1) * NP, tb * QB : (tb + 1) * QB]
                )
                xts.append(xtile)
            psq = projps.tile([NP, QB], F32, tag="proj")
            for d in range(NDC):
                nc.tensor.matmul(
                    psq[:], wqc[:, d, :], xts[d][:],
                    start=(d == 0), stop=(d == NDC - 1),
                )
            nc.scalar.activation(
                qt[:, tb * QB : (tb + 1) * QB], psq[:], Identity,
                bias=bq_sb[:, c : c + 1],
            )
            psk = projps.tile([NP, QB], F32, tag="proj")
            for d in range(NDC):
                nc.tensor.matmul(
                    psk[:], wkc[:, d, :], xts[d][:],
                    start=(d == 0), stop=(d == NDC - 1),
                )
            nc.scalar.activation(
                kt_t[:, tb * QB : (tb + 1) * QB], psk[:], Identity,
                bias=bk_sb[:, c : c + 1],
            )

        # attention for this pair
        for qb in range(NTB):
            nkt = 4 * qb + 4  # k-tiles 0..4qb+3 (last 4 are diagonal)
            pv = [
                pvps.tile([DK + 1, QB], F32, tag=f"pv{h}", name=f"pv{h}")
                for h in range(2)
            ]
            for kti in range(nkt):
                di = kti - 4 * qb  # >=0 on diagonal tiles
                o = max(di, 0) * NP  # first causally valid column
                sps = [
                    scoreps.tile([NP, QB], F32, tag=f"s{h}", name=f"s{h}")
                    for h in range(2)
                ]
                for h in range(2):
                    nc.tensor.matmul(
                        sps[h][:, o:QB],
                        kt_t[64 * h : 64 * h + 64, kti * NP : (kti + 1) * NP],
                        qt[64 * h : 64 * h + 64, qb * QB + o : (qb + 1) * QB],
                        start=True, stop=True,
                        tile_position=(64 * h, 0),
                    )
                for h in range(2):
                    et = expp.tile([NP, QB], BF16, tag=f"e{h}", name=f"e{h}")
                    if di >= 0:
                        tmp = expp.tile([NP, QB], F32, tag="tmp")
                        nc.vector.tensor_add(
                            tmp[:, o:QB], sps[h][:, o:QB], msk_sb[:, di, o:QB]
                        )
                        nc.scalar.activation(
                            et[:, o:QB], tmp[:, o:QB], Exp, scale=0.125
                        )
                    else:
                        nc.scalar.activation(
                            et[:, o:QB], sps[h][:, o:QB], Exp, scale=0.125
                        )
                    hh = 2 * c + h
                    nc.tensor.matmul(
                        pv[h][:, o:QB],
                        v_sb[kti][:, hh, :],
                        et[:, o:QB],
                        start=(kti == 0), stop=(kti == nkt - 1),
                    )
            for h in range(2):
                recip = smallp.tile([1, QB], F32, tag="recip")
                nc.vector.reciprocal(recip[:], pv[h][DK : DK + 1, :])
                bc = projps.tile([NP, QB], F32, tag="proj")
                nc.tensor.matmul(
                    bc[0:DK, :], ones_f[0:1, 0:DK], recip[:],
                    start=True, stop=True,
                )
                bcs = smallp.tile([DK, QB], F32, tag="bcs")
                nc.vector.tensor_copy(bcs[:], bc[0:DK, :])
                nc.vector.tensor_mul(
                    oat[c][64 * h : 64 * h + 64, qb * QB : (qb + 1) * QB],
                    pv[h][0:DK, :],
                    bcs[:],
                )

    # ---- final projection: y[t, dout] = outAllT.T @ Wo ----
    wop = ctx.enter_context(tc.tile_pool(name="wop", bufs=1))
    wo_sb = wop.tile([NP, NDC // 2, D], BF16, tag="wo")
    for c in range(NPAIR):
        nc.sync.dma_start(wo_sb[:, c, :], wo[c * NP : (c + 1) * NP, :])
    for tt in range(NTT):
        for dh in range(2):
            ps = projps.tile([NP, QB], F32, tag="proj")
            for c in range(NPAIR):
                nc.tensor.matmul(
                    ps[:],
                    oat[c][:, tt * NP : (tt + 1) * NP],
                    wo_sb[:, c, dh * QB : (dh + 1) * QB],
                    start=(c == 0), stop=(c == NPAIR - 1),
                )
            ystage = smallp.tile([NP, QB], F32, tag="ystage", bufs=2)
            nc.vector.tensor_copy(ystage[:], ps[:])
            nc.sync.dma_start(
                y[tt * NP : (tt + 1) * NP, dh * QB : (dh + 1) * QB], ystage[:]
            )


def _install_ntff_hook_shim():
    """The agent image's antenv lacks axon_hooks, so trace=True under axon
    degrades. Provide the missing module and register the ctypes NTFF hook
    from trn_agent_boot. Best-effort: failures just mean no trace."""
    try:
        import sys
        import types

        if "antenv.axon_hooks" not in sys.modules:
            mod = types.ModuleType("antenv.axon_hooks")
            mod._hook = None
            mod.set_axon_ntff_profile_hook = lambda h: setattr(mod, "_hook", h)
            mod.get_axon_ntff_profile_hook = lambda: mod._hook
            sys.modules["antenv.axon_hooks"] = mod
            import antenv

            antenv.axon_hooks = mod
        from antenv.axon_hooks import (
            get_axon_ntff_profile_hook,
            set_axon_ntff_profile_hook,
        )

        if get_axon_ntff_profile_hook() is None:
            from trn_agent_boot.trn_boot import _ntff_profile_via_ctypes

            hook = _ntff_profile_via_ctypes("/opt/axon/libaxon_pjrt.so")
            if hook is not None:
                set_axon_ntff_profile_hook(hook)
    except Exception as e:  # noqa: BLE001
        print(f"ntff hook shim failed ({e}); running without trace")


def _round_f32r(a: np.ndarray) -> np.ndarray:
    """Round fp32 to fp32r (11-bit mantissa, low 12 bits zero), RNE."""
    u = np.ascontiguousarray(a, dtype=np.float32).view(np.uint32)
    u = (u + np.uint32(0x7FF) + ((u >> np.uint32(12)) & np.uint32(1))) & np.uint32(
        0xFFFFF000
    )
    return u.view(np.float32)


def _qdt(a: np.ndarray) -> np.ndarray:
    if MODE == "fast":
        return np.ascontiguousarray(a, dtype=np.float32).astype(ml_dtypes.bfloat16)
    return _round_f32r(a)


def _bf(a: np.ndarray) -> np.ndarray:
    return np.ascontiguousarray(a, dtype=np.float32).astype(ml_dtypes.bfloat16)


def _make_masks() -> np.ndarray:
    m = np.zeros((4, NP, QB), dtype=np.float32)
    kk = np.arange(NP)[:, None]
    qq = np.arange(QB)[None, :]
    for o in range(4):
        m[o] = np.where(qq >= kk + o * NP, 0.0, -1e30)
    return m


def kernel(x, Wq, bq, Wk, bk, Wv, bv, Wo, bo):
    x = np.ascontiguousarray(np.asarray(x, dtype=np.float32))
    Wq, bq = np.asarray(Wq, np.float32), np.asarray(bq, np.float32)
    Wk, bk = np.asarray(Wk, np.float32), np.asarray(bk, np.float32)
    Wv, bv = np.asarray(Wv, np.float32), np.asarray(bv, np.float32)
    Wo, bo = np.asarray(Wo, np.float32), np.asarray(bo, np.float32)

    if "nc" not in _CACHE:
        _CACHE["nc"] = _build_nc()
    nc = _CACHE["nc"]

    masks = _make_masks()
    ones_bf = np.ones((NP, QB), dtype=ml_dtypes.bfloat16)
    in_maps = []
    for core in range(8):
        b, hg = core // 2, core % 2
        cs = slice(hg * CD, (hg + 1) * CD)
        in_maps.append(
            {
                "xt": _qdt(x[b].T),
                "wq": _qdt(Wq[:, cs]),
                "wk": _qdt(Wk[:, cs]),
                "wv": _bf(Wv[:, cs]),
                "wo": _bf(Wo[cs, :]),
                "bqc": np.ascontiguousarray(bq[cs].reshape(NPAIR, NP).T),
                "bkc": np.ascontiguousarray(bk[cs].reshape(NPAIR, NP).T),
                "bvr": _bf(bv[cs].reshape(1, CD)),
                "msk": masks,
                "onesd": ones_bf,
            }
        )

    trace = bool(os.environ.get("KERNEL_TRACE"))
    if trace:
        _install_ntff_hook_shim()
    res = run_bass_kernel_spmd(
        nc, in_maps, core_ids=list(range(8)), trace=trace
    )
    _CACHE["last_results"] = res

    out = np.empty((B, T, D), dtype=np.float32)
    for b in range(B):
        out[b] = res.results[2 * b]["y"] + res.results[2 * b + 1]["y"] + bo
    return out



# revision 7
# speedup vs baseline: 1.7199x; 1.7199x over previous
"""Causal self-attention (B=4, T=2048, D=1024, H=16) on 8 NeuronCores - v2.

Sharding: core c handles batch b=c//2 and head-group hg=c%2 (8 of 16 heads,
processed as 4 head pairs). Column-parallel Wq/Wk/Wv (512 cols), row-parallel
Wo (512 rows). Host sums the two partial outputs per batch and adds bo.

All-bf16 PE compute, fp32 PSUM accumulation. x^T and every weight is loaded
into SBUF once and stays resident (no per-pair re-reads). The k-side bias is
dropped: (q+bq).(k+bk) differs from (q+bq).k by a per-query constant, which
softmax cancels exactly. The q bias is folded into the projection as a K=1
rank-1 matmul.

Attention per (pair, query-block): score matmuls row-packed 2 heads via
tile_position; ONE exp per k-tile covering both heads ([128, 2, 512] PSUM
tile spanning 2 banks); causal masking on diagonal k-tiles via a post-exp
multiply with a 128x128 triangular 0/1 mask (only the one partial subtile);
PV with a ones column producing sumexp. Normalization: sumexp rows gathered
by DVE copies into [8, 512], one batched reciprocal per pair, broadcast to
[128, 512] via a bf16 K=8 selection matmul, applied with one DVE multiply.

Q/K projections of pair c+1 are emitted after attention of pair c so the Tile
scheduler fills PE bubbles with projection matmuls (keeps HAM at 2.4 GHz).
"""

import os
from contextlib import ExitStack

import ml_dtypes
import numpy as np

import concourse.bacc as bacc
import concourse.mybir as mybir
import concourse.tile as tile
from concourse.bass_utils import run_bass_kernel_spmd

B, T, D, H, DK = 4, 2048, 1024, 16, 64
HL = 8  # heads per core
CD = HL * DK  # 512 local channels
NP = 128
QB = 512
NDC = D // NP  # 8 din chunks
NTT = T // NP  # 16 t-tiles
NTB = T // QB  # 4 t-blocks
NPAIR = HL // 2  # 4 head pairs
F32 = mybir.dt.float32
BF16 = mybir.dt.bfloat16
Exp = mybir.ActivationFunctionType.Exp

_CACHE: dict = {}


def _build_nc():
    nc = bacc.Bacc("TRN2", target_bir_lowering=False, debug=False)
    xt = nc.dram_tensor("xt", [D, T], BF16, kind="ExternalInput")
    wq = nc.dram_tensor("wq", [D, CD], BF16, kind="ExternalInput")
    wk = nc.dram_tensor("wk", [D, CD], BF16, kind="ExternalInput")
    wv = nc.dram_tensor("wv", [D, CD], BF16, kind="ExternalInput")
    wo = nc.dram_tensor("wo", [CD, D], BF16, kind="ExternalInput")
    bqr = nc.dram_tensor("bqr", [1, CD], BF16, kind="ExternalInput")
    bvr = nc.dram_tensor("bvr", [1, CD], BF16, kind="ExternalInput")
    tri = nc.dram_tensor("tri", [NP, 2, NP], BF16, kind="ExternalInput")
    sel = nc.dram_tensor("sel", [8, NTB, NP], BF16, kind="ExternalInput")
    y = nc.dram_tensor("y", [T, D], F32, kind="ExternalOutput")

    with tile.TileContext(nc) as tc, ExitStack() as ctx:
        _body(nc, tc, ctx, xt, wq, wk, wv, wo, bqr, bvr, tri, sel, y)
    nc.compile()
    return nc


def _body(nc, tc, ctx, xt, wq, wk, wv, wo, bqr, bvr, tri, sel, y):
    const = ctx.enter_context(tc.tile_pool(name="const", bufs=1))
    wpool = ctx.enter_context(tc.tile_pool(name="w", bufs=1))
    xpool = ctx.enter_context(tc.tile_pool(name="x", bufs=1))
    vpool = ctx.enter_context(tc.tile_pool(name="v", bufs=1))
    oatp = ctx.enter_context(tc.tile_pool(name="oat", bufs=1))
    qkp = ctx.enter_context(tc.tile_pool(name="qk", bufs=2))
    etp = ctx.enter_context(tc.tile_pool(name="et", bufs=4))
    zp = ctx.enter_context(tc.tile_pool(name="z", bufs=2))
    ystp = ctx.enter_context(tc.tile_pool(name="yst", bufs=4))
    # PSUM: proj 2 banks + score 2x2 banks + pv 2 banks = 8
    projps = ctx.enter_context(tc.tile_pool(name="projps", bufs=2, space="PSUM"))
    scoreps = ctx.enter_context(tc.tile_pool(name="scoreps", bufs=2, space="PSUM"))
    pvps = ctx.enter_context(tc.tile_pool(name="pvps", bufs=1, space="PSUM"))

    # constants
    ones_t = const.tile([1, QB], BF16, tag="ones")
    nc.vector.memset(ones_t[:], 1.0)
    bq_sb = const.tile([1, CD], BF16, tag="bq")
    nc.sync.dma_start(bq_sb[:], bqr[:])
    bv_sb = const.tile([1, CD], BF16, tag="bv")
    nc.sync.dma_start(bv_sb[:], bvr[:])
    tri_sb = const.tile([NP, 2, NP], BF16, tag="tri")
    nc.sync.dma_start(tri_sb[:], tri[:])
    sel_sb = const.tile([8, NTB, NP], BF16, tag="sel")
    nc.sync.dma_start(sel_sb[:], sel[:])
    # warm up the exp table set early (one-time ~2.7us load overlaps V phase)
    warm = const.tile([1, 2], F32, tag="warm")
    nc.vector.memset(warm[:], 0.0)
    nc.scalar.activation(warm[:], warm[:], Exp)

    # resident x^T and weights
    xt_sb = xpool.tile([NP, NDC, T], BF16, tag="xt")
    for tb in range(NTB):
        for d in range(NDC):
            nc.sync.dma_start(
                xt_sb[:, d, tb * QB : (tb + 1) * QB],
                xt[d * NP : (d + 1) * NP, tb * QB : (tb + 1) * QB],
            )
    wv_sb = wpool.tile([NP, NDC, CD], BF16, tag="wv")
    wq_sb = wpool.tile([NP, NDC, CD], BF16, tag="wq")
    wk_sb = wpool.tile([NP, NDC, CD], BF16, tag="wk")
    for d in range(NDC):
        nc.sync.dma_start(wv_sb[:, d, :], wv[d * NP : (d + 1) * NP, :])
    for d in range(NDC):
        nc.sync.dma_start(wq_sb[:, d, :], wq[d * NP : (d + 1) * NP, :])
        nc.sync.dma_start(wk_sb[:, d, :], wk[d * NP : (d + 1) * NP, :])
    wo_sb = wpool.tile([NP, NPAIR, D], BF16, tag="wo")
    for cc in range(NPAIR):
        nc.sync.dma_start(wo_sb[:, cc, :], wo[cc * NP : (cc + 1) * NP, :])

    # ---- V phase: V[t, 8 heads x (64 dv + 1 ones col)] ----
    v_sb = [
        vpool.tile([NP, HL, DK + 1], BF16, tag=f"v{tt}", name=f"v{tt}")
        for tt in range(NTT)
    ]
    for tb in range(NTB):
        for i in range(QB // NP):
            tt = tb * (QB // NP) + i
            ps = projps.tile([NP, CD], F32, tag="proj")
            for d in range(NDC):
                nc.tensor.matmul(
                    ps[:],
                    xt_sb[:, d, tt * NP : (tt + 1) * NP],
                    wv_sb[:, d, :],
                    start=(d == 0),
                    stop=False,
                )
            nc.tensor.matmul(
                ps[:], ones_t[0:1, 0:NP], bv_sb[:], start=False, stop=True
            )
            vt = v_sb[tt]
            nc.vector.memset(vt[:, :, DK : DK + 1], 1.0)
            nc.vector.tensor_copy(vt[:, :, 0:DK], ps.rearrange("p (h k) -> p h k", h=HL))

    oat = [oatp.tile([NP, T], BF16, tag=f"oat{c}", name=f"oat{c}") for c in range(NPAIR)]

    def proj(c):
        qt = qkp.tile([NP, T], BF16, tag="qt", name=f"qt{c}")
        kt = qkp.tile([NP, T], BF16, tag="kt", name=f"kt{c}")
        for tb in range(NTB):
            sl = slice(tb * QB, (tb + 1) * QB)
            psq = projps.tile([NP, QB], F32, tag="proj")
            for d in range(NDC):
                nc.tensor.matmul(
                    psq[:], wq_sb[:, d, c * NP : (c + 1) * NP], xt_sb[:, d, sl],
                    start=(d == 0), stop=False,
                )
            nc.tensor.matmul(
                psq[:], bq_sb[0:1, c * NP : (c + 1) * NP], ones_t[:],
                start=False, stop=True,
            )
            nc.vector.tensor_copy(qt[:, sl], psq[:])
            psk = projps.tile([NP, QB], F32, tag="proj")
            for d in range(NDC):
                nc.tensor.matmul(
                    psk[:], wk_sb[:, d, c * NP : (c + 1) * NP], xt_sb[:, d, sl],
                    start=(d == 0), stop=(d == NDC - 1),
                )
            nc.vector.tensor_copy(kt[:, sl], psk[:])
        return qt, kt

    def attn(c, qt, kt):
        zsb = zp.tile([1, 8, QB], F32, tag="zsb", name=f"zsb{c}")
        for qb in range(NTB):
            nkt = 4 * qb + 4
            pv = pvps.tile([DK + 1, 2, QB], F32, tag="pv")
            for kti in range(nkt):
                di = kti - 4 * qb
                o = max(di, 0) * NP
                sps = scoreps.tile([NP, 2, QB], F32, tag="s")
                for h in range(2):
                    nc.tensor.matmul(
                        sps[:, h, o:QB],
                        kt[64 * h : 64 * h + 64, kti * NP : (kti + 1) * NP],
                        qt[64 * h : 64 * h + 64, qb * QB + o : (qb + 1) * QB],
                        start=True, stop=True,
                        tile_position=(64 * h, 0),
                    )
                et = etp.tile([NP, 2, QB], BF16, tag="et")
                nc.scalar.activation(et[:, :, o:QB], sps[:, :, o:QB], Exp, scale=0.125)
                if di >= 0:
                    nc.vector.tensor_mul(
                        et[:, :, o : o + NP], et[:, :, o : o + NP], tri_sb[:]
                    )
                for h in range(2):
                    nc.tensor.matmul(
                        pv[:, h, o:QB],
                        v_sb[kti][:, 2 * c + h, :],
                        et[:, h, o:QB],
                        start=(kti == 0), stop=(kti == nkt - 1),
                    )
            for h in range(2):
                r = 2 * qb + h
                nc.vector.tensor_copy(zsb[0:1, r, :], pv[DK : DK + 1, h, :])
                nc.vector.tensor_copy(
                    oat[c][64 * h : 64 * h + 64, qb * QB : (qb + 1) * QB],
                    pv[0:DK, h, :],
                )
        return zsb

    def norm(c, zsb):
        # scatter the 8 sumexp rows to 8 partitions (DMA is partition-free)
        za = zp.tile([8, QB], F32, tag="za", name=f"za{c}")
        nc.sync.dma_start(za[:], zsb[0:1, :, :])
        zr = zp.tile([8, QB], BF16, tag="zr", name=f"zr{c}")
        with nc.allow_low_precision(reason="1/sumexp broadcast in bf16 is intentional"):
            nc.vector.reciprocal(zr[:], za[:])
        for g in range(2):
            bc = scoreps.tile([NP, 2, QB], F32, tag="s")
            for j in range(2):
                qb = 2 * g + j
                nc.tensor.matmul(
                    bc[:, j, :], sel_sb[:, qb, :], zr[:], start=True, stop=True
                )
                nc.vector.tensor_mul(
                    oat[c][:, qb * QB : (qb + 1) * QB],
                    oat[c][:, qb * QB : (qb + 1) * QB],
                    bc[:, j, :],
                )

    qts = proj(0)
    for c in range(NPAIR):
        za = attn(c, *qts)
        if c + 1 < NPAIR:
            qts = proj(c + 1)
        norm(c, za)

    # ---- final projection: y[t, dout] = oat.T @ Wo ----
    for tt in range(NTT):
        pss = [
            projps.tile([NP, QB], F32, tag="proj", name=f"yps{dh}") for dh in range(2)
        ]
        for cc in range(NPAIR):
            for dh in range(2):
                nc.tensor.matmul(
                    pss[dh][:],
                    oat[cc][:, tt * NP : (tt + 1) * NP],
                    wo_sb[:, cc, dh * QB : (dh + 1) * QB],
                    start=(cc == 0), stop=(cc == NPAIR - 1),
                )
        for dh in range(2):
            ys = ystp.tile([NP, QB], F32, tag="ys")
            nc.vector.tensor_copy(ys[:], pss[dh][:])
            nc.sync.dma_start(
                y[tt * NP : (tt + 1) * NP, dh * QB : (dh + 1) * QB], ys[:]
            )


def _install_ntff_hook_shim():
    """The agent image's antenv lacks axon_hooks, so trace=True under axon
    degrades. Provide the missing module and register the ctypes NTFF hook
    from trn_agent_boot. Best-effort: failures just mean no trace."""
    try:
        import sys
        import types

        if "antenv.axon_hooks" not in sys.modules:
            mod = types.ModuleType("antenv.axon_hooks")
            mod._hook = None
            mod.set_axon_ntff_profile_hook = lambda h: setattr(mod, "_hook", h)
            mod.get_axon_ntff_profile_hook = lambda: mod._hook
            sys.modules["antenv.axon_hooks"] = mod
            import antenv

            antenv.axon_hooks = mod
        from antenv.axon_hooks import (
            get_axon_ntff_profile_hook,
            set_axon_ntff_profile_hook,
        )

        if get_axon_ntff_profile_hook() is None:
            from trn_agent_boot.trn_boot import _ntff_profile_via_ctypes

            hook = _ntff_profile_via_ctypes("/opt/axon/libaxon_pjrt.so")
            if hook is not None:
                set_axon_ntff_profile_hook(hook)
    except Exception as e:  # noqa: BLE001
        print(f"ntff hook shim failed ({e}); running without trace")


def _bf(a: np.ndarray) -> np.ndarray:
    return np.ascontiguousarray(a, dtype=np.float32).astype(ml_dtypes.bfloat16)


def _make_tri() -> np.ndarray:
    t = (np.arange(NP)[None, :] >= np.arange(NP)[:, None]).astype(np.float32)
    return _bf(np.stack([t, t], axis=1))


def _make_sel() -> np.ndarray:
    s = np.zeros((8, NTB, NP), dtype=np.float32)
    for qb in range(NTB):
        s[2 * qb, qb, 0:DK] = 1.0
        s[2 * qb + 1, qb, DK:NP] = 1.0
    return _bf(s)


def kernel(x, Wq, bq, Wk, bk, Wv, bv, Wo, bo):
    x = np.ascontiguousarray(np.asarray(x, dtype=np.float32))
    Wq, bq = np.asarray(Wq, np.float32), np.asarray(bq, np.float32)
    Wk = np.asarray(Wk, np.float32)
    Wv, bv = np.asarray(Wv, np.float32), np.asarray(bv, np.float32)
    Wo, bo = np.asarray(Wo, np.float32), np.asarray(bo, np.float32)

    if "nc" not in _CACHE:
        _CACHE["nc"] = _build_nc()
    nc = _CACHE["nc"]

    tri = _make_tri()
    selm = _make_sel()
    in_maps = []
    for core in range(8):
        b, hg = core // 2, core % 2
        cs = slice(hg * CD, (hg + 1) * CD)
        in_maps.append(
            {
                "xt": _bf(x[b].T),
                "wq": _bf(Wq[:, cs]),
                "wk": _bf(Wk[:, cs]),
                "wv": _bf(Wv[:, cs]),
                "wo": _bf(Wo[cs, :]),
                "bqr": _bf(bq[cs].reshape(1, CD)),
                "bvr": _bf(bv[cs].reshape(1, CD)),
                "tri": tri,
                "sel": selm,
            }
        )

    trace = bool(os.environ.get("KERNEL_TRACE"))
    if trace:
        _install_ntff_hook_shim()
    res = run_bass_kernel_spmd(
        nc, in_maps, core_ids=list(range(8)), trace=trace
    )
    _CACHE["last_results"] = res

    out = np.empty((B, T, D), dtype=np.float32)
    for b in range(B):
        out[b] = res.results[2 * b]["y"] + res.results[2 * b + 1]["y"] + bo
    return out
